# revision 17
# baseline (speedup 1.0000x reference)
"""MoE top-2 routing kernel for 8 Trainium2 NeuronCores.

Strategy (expert-parallel, per the sharding hint):
  - Host computes the (tiny) router in float64: logits -> softmax -> top-2 ->
    renormalize.  67 MFLOP total, ~0.05% of the model FLOPs.  Selection was
    verified tie-safe: min prob gap between 2nd/3rd expert is ~8e-6 while
    cross-backend fp32 logit noise is ~3e-7.
  - Tokens are dispatched (gathered) per expert on the host; expert e's token
    batch, padded to capacity C, goes to core e along with expert e's weights
    (pre-transposed on host into the exact SBUF-friendly layouts the kernel
    consumes, cast to bf16).
  - Each core runs a dense FFN over its C tokens:
        hT = gelu(w1T.T-contractions)   (PSUM fp32 accum, bias fused in ACT)
        oT = w2-contractions over hT
    with D/H features on the partition axis end-to-end, so no on-device
    transposes are needed anywhere.
  - Host applies the combine weights and scatter-adds per-expert outputs back
    into the full [B,S,D] output (each token appears in exactly 2 experts).

Per-core layouts (E=8 experts, D=1024, H=4096, C tokens):
  xT  [128, 8, C]        bf16   xT[p, ko, t]       = x_g[t, ko*128+p]
  w1t [128, 32, 8, 128]  bf16   w1t[p, ho, ko, j]  = w1[e][ho*128+j, ko*128+p]
  w2t [128, 8, 32, 128]  bf16   w2t[p, do, ko, j]  = w2[e][do*128+j, ko*128+p]
  b1t [128, 32]          f32    b1t[p, ho]         = b1[e][ho*128+p]
  oT  [128, 8, C]        f32    oT[p, do, t]       = o_g[t, do*128+p]
"""

import numpy as np
import ml_dtypes

TOP_K = 2
P = 128
D = 1024
H = 4096
E = 8

_COMPILED = {}  # C -> compiled Bacc instance


def _token_blocks(C):
    """Split C into blocks <=512, each >=256 (so LDWEIGHTS stays hidden)."""
    nblk = -(-C // 512)
    base = C // nblk
    sizes = []
    rem = C
    for i in range(nblk):
        s = -(-rem // (nblk - i))
        # round up to /32 except last
        if i < nblk - 1:
            s = min(512, ((s + 31) // 32) * 32)
        sizes.append(s)
        rem -= s
    assert sum(sizes) == C and all(s <= 512 for s in sizes), sizes
    return sizes


def _build_ffn_kernel(C):
    import concourse.mybir as mybir
    import concourse.tile as tile
    from concourse import bacc

    blocks = _token_blocks(C)
    starts = [sum(blocks[:i]) for i in range(len(blocks))]
    NTMAX = max(blocks)
    bf16 = mybir.dt.bfloat16
    f32 = mybir.dt.float32

    nc = bacc.Bacc("TRN2", target_bir_lowering=False, debug=False)
    # x is stored block-major: per partition, the [D//P, sz] slab of each
    # token block is contiguous, so each block is one clean DMA.
    xT = nc.dram_tensor("xT", [P, D // P * C], bf16, kind="ExternalInput").ap()
    w1t = nc.dram_tensor("w1t", [P, H // P, D // P, P], bf16, kind="ExternalInput").ap()
    w2t = nc.dram_tensor("w2t", [P, D // P, H // P, P], bf16, kind="ExternalInput").ap()
    b1t = nc.dram_tensor("b1t", [P, H // P], f32, kind="ExternalInput").ap()
    oT = nc.dram_tensor("oT", [P, D // P, C], f32, kind="ExternalOutput").ap()

    with tile.TileContext(nc) as tc:
        with (
            tc.tile_pool(name="const", bufs=1) as cpool,
            tc.tile_pool(name="resident", bufs=1) as rpool,
            tc.tile_pool(name="warm", bufs=1) as warmpool,
            tc.tile_pool(name="w1p", bufs=4) as w1pool,
            tc.tile_pool(name="w2p", bufs=3) as w2pool,
            tc.tile_pool(name="ost", bufs=4) as opool,
            tc.tile_pool(name="ps", bufs=4, space="PSUM") as pspool,
            tc.tile_pool(name="wps", bufs=1, space="PSUM") as wpspool,
        ):
            # PE warm-up: dependency-free matmuls run while input DMAs are in
            # flight, so the HAM clock-gate is already at 8/8 when real
            # matmuls start.  Zeroed input avoids NaN notifications.
            wsrc = warmpool.tile([P, 512], bf16)
            nc.gpsimd.memset(wsrc[:], 0.0)
            wps = wpspool.tile([P, 512], f32)
            for _ in range(7):
                nc.tensor.matmul(wps[:], wsrc[:, :P], wsrc[:], start=True, stop=True)

            # x arrives block-contiguous, split across THREE DMA channels so
            # the first-needed transfers run in parallel:
            #   GpSimd(SWDGE): x block 0 (earliest issue, gates the first MM)
            #   Sync  (Q1):    w1 slab 0, x block 2, w1 slabs 1..
            #   Scalar(Q10):   x block 1, b1, w2 prefetch
            w1_first = w1pool.tile([P, D // P, P], bf16, tag="w1s")
            nc.sync.dma_start(w1_first[:], w1t[:, 0])
            x_engines = [nc.gpsimd, nc.scalar, nc.sync]
            x_blks = []
            for blk, (st, sz) in enumerate(zip(starts, blocks)):
                xb = rpool.tile([P, D // P * sz], bf16, tag=f"xb{blk}")
                eng = x_engines[blk % 3]
                eng.dma_start(xb[:], xT[:, D // P * st : D // P * (st + sz)])
                x_blks.append(xb)
            b1_sb = cpool.tile([P, H // P], f32)
            nc.scalar.dma_start(b1_sb[:], b1t[:])
            h_sb = rpool.tile([P, H // P, C], bf16)

            # Layer 1: hT[:, ho, t] = gelu(sum_ko w1t[:,ho,ko,:].T @ xT[:,ko,t] + b1)
            for ho in range(H // P):
                if ho == 0:
                    w1s = w1_first
                else:
                    w1s = w1pool.tile([P, D // P, P], bf16, tag="w1s")
                    nc.sync.dma_start(w1s[:], w1t[:, ho])
                for blk, (st, sz) in enumerate(zip(starts, blocks)):
                    ps = pspool.tile([P, NTMAX], f32, tag="ps")
                    for ko in range(D // P):
                        nc.tensor.matmul(
                            ps[:, :sz],
                            w1s[:, ko, :],
                            x_blks[blk][:, ko * sz : (ko + 1) * sz],
                            start=(ko == 0),
                            stop=(ko == D // P - 1),
                        )
                    nc.scalar.activation(
                        h_sb[:, ho, st : st + sz],
                        ps[:, :sz],
                        mybir.ActivationFunctionType.Gelu,
                        bias=b1_sb[:, ho : ho + 1],
                    )

            # Layer 2: oT[:, do, t] = sum_ko w2t[:,do,ko,:].T @ hT[:,ko,t]
            for do in range(D // P):
                w2s = w2pool.tile([P, H // P, P], bf16, tag="w2s")
                nc.scalar.dma_start(w2s[:], w2t[:, do])
                for bi, (st, sz) in enumerate(zip(starts, blocks)):
                    ps = pspool.tile([P, NTMAX], f32, tag="ps")
                    for ko in range(H // P):
                        nc.tensor.matmul(
                            ps[:, :sz],
                            w2s[:, ko, :],
                            h_sb[:, ko, st : st + sz],
                            start=(ko == 0),
                            stop=(ko == H // P - 1),
                        )
                    last = do == D // P - 1 and bi == len(blocks) - 1
                    if not last:
                        ob = opool.tile([P, NTMAX], f32, tag="ob")
                        nc.vector.tensor_copy(ob[:, :sz], ps[:, :sz])
                        nc.scalar.dma_start(oT[:, do, st : st + sz], ob[:, :sz])
                    else:
                        # Final eviction is on the critical path: split it so
                        # the first half's DMA overlaps the second half's copy,
                        # using both HWDGE queues.
                        hsz = sz // 2
                        ob = opool.tile([P, NTMAX], f32, tag="ob")
                        nc.vector.tensor_copy(ob[:, :hsz], ps[:, :hsz])
                        nc.sync.dma_start(oT[:, do, st : st + hsz], ob[:, :hsz])
                        nc.vector.tensor_copy(ob[:, hsz:sz], ps[:, hsz:sz])
                        nc.scalar.dma_start(
                            oT[:, do, st + hsz : st + sz], ob[:, hsz:sz]
                        )

    nc.compile()
    return nc


def _route_host(x_flat, router_w):
    """Float64 router: returns per-expert (token_idx, combine_weight)."""
    logits = x_flat.astype(np.float64) @ router_w.astype(np.float64).T
    m = logits.max(axis=-1, keepdims=True)
    p = np.exp(logits - m)
    p /= p.sum(axis=-1, keepdims=True)
    order = np.argsort(-p, axis=-1)
    topi = order[:, :TOP_K]
    topw = np.take_along_axis(p, topi, axis=-1)
    topw /= topw.sum(axis=-1, keepdims=True)

    idx_list, wgt_list = [], []
    for e in range(E):
        mask = topi == e  # [T, TOP_K]; at most one True per row
        rows = np.nonzero(mask.any(axis=-1))[0]
        w = topw[rows][mask[rows]]
        idx_list.append(rows)
        wgt_list.append(w.astype(np.float32))
    return idx_list, wgt_list


def kernel(x, router_w, w1, b1, w2, b2):
    from concourse import bass_utils

    x = np.asarray(x)
    router_w = np.asarray(router_w)
    w1 = np.asarray(w1)
    b1 = np.asarray(b1)
    w2 = np.asarray(w2)
    b2 = np.asarray(b2)

    B, S, _ = x.shape
    T = B * S
    x_flat = x.reshape(T, D)

    idx_list, wgt_list = _route_host(x_flat, router_w)
    max_cnt = max(len(i) for i in idx_list)
    C = ((max_cnt + 31) // 32) * 32

    if C not in _COMPILED:
        _COMPILED[C] = _build_ffn_kernel(C)
    nc = _COMPILED[C]

    blocks = _token_blocks(C)
    starts = [sum(blocks[:i]) for i in range(len(blocks))]
    bf = ml_dtypes.bfloat16
    in_maps = []
    for e in range(E):
        idx = idx_list[e]
        n_e = len(idx)
        # xT [128, 8*C] block-major: pad tokens to C with zeros
        xg = np.zeros((C, D), np.float32)
        xg[:n_e] = x_flat[idx]
        xT_full = xg.T.reshape(D // P, P, C).transpose(1, 0, 2)  # [128, 8, C]
        xT_d = np.concatenate(
            [xT_full[:, :, st : st + sz].reshape(P, -1) for st, sz in zip(starts, blocks)],
            axis=1,
        ).astype(bf)
        w1_d = np.ascontiguousarray(
            w1[e].reshape(H // P, P, D // P, P).transpose(3, 0, 2, 1)
        ).astype(bf)
        w2_d = np.ascontiguousarray(
            w2[e].reshape(D // P, P, H // P, P).transpose(3, 0, 2, 1)
        ).astype(bf)
        b1_d = np.ascontiguousarray(b1[e].reshape(H // P, P).T).astype(np.float32)
        in_maps.append({"xT": xT_d, "w1t": w1_d, "w2t": w2_d, "b1t": b1_d})

    res = bass_utils.run_bass_kernel_spmd(nc, in_maps, core_ids=list(range(E)))

    out = np.zeros((T, D), np.float32)
    for e in range(E):
        idx = idx_list[e]
        n_e = len(idx)
        oT = res.results[e]["oT"]  # [128, 8, C]
        o_g = oT.transpose(1, 0, 2).reshape(D, C)[:, :n_e].T  # [n_e, D]
        out[idx] += wgt_list[e][:, None] * (o_g + b2[e][None, :])
    return out.reshape(B, S, D).astype(np.float32)


# revision 19
# speedup vs baseline: 1.0209x; 1.0209x over previous
"""MoE top-2 routing kernel for 8 Trainium2 NeuronCores.

Strategy (expert-parallel, per the sharding hint):
  - Host computes the (tiny) router in float64: logits -> softmax -> top-2 ->
    renormalize.  67 MFLOP total, ~0.05% of the model FLOPs.  Selection was
    verified tie-safe: min prob gap between 2nd/3rd expert is ~8e-6 while
    cross-backend fp32 logit noise is ~3e-7.
  - Tokens are dispatched (gathered) per expert on the host; expert e's token
    batch, padded to capacity C, goes to core e along with expert e's weights
    (pre-transposed on host into the exact SBUF-friendly layouts the kernel
    consumes, cast to bf16).
  - Each core runs a dense FFN over its C tokens:
        hT = gelu(w1T.T-contractions)   (PSUM fp32 accum, bias fused in ACT)
        oT = w2-contractions over hT
    with D/H features on the partition axis end-to-end, so no on-device
    transposes are needed anywhere.
  - Host applies the combine weights and scatter-adds per-expert outputs back
    into the full [B,S,D] output (each token appears in exactly 2 experts).

Per-core layouts (E=8 experts, D=1024, H=4096, C tokens):
  xT  [128, 8, C]        bf16   xT[p, ko, t]       = x_g[t, ko*128+p]
  w1t [128, 32, 8, 128]  bf16   w1t[p, ho, ko, j]  = w1[e][ho*128+j, ko*128+p]
  w2t [128, 8, 32, 128]  bf16   w2t[p, do, ko, j]  = w2[e][do*128+j, ko*128+p]
  b1t [128, 32]          f32    b1t[p, ho]         = b1[e][ho*128+p]
  oT  [128, 8, C]        f32    oT[p, do, t]       = o_g[t, do*128+p]
"""

import numpy as np
import ml_dtypes

TOP_K = 2
P = 128
D = 1024
H = 4096
E = 8

_COMPILED = {}  # C -> compiled Bacc instance


def _token_blocks(C):
    """Split C into blocks <=512, each >=256 (so LDWEIGHTS stays hidden)."""
    nblk = -(-C // 512)
    base = C // nblk
    sizes = []
    rem = C
    for i in range(nblk):
        s = -(-rem // (nblk - i))
        # round up to /32 except last
        if i < nblk - 1:
            s = min(512, ((s + 31) // 32) * 32)
        sizes.append(s)
        rem -= s
    assert sum(sizes) == C and all(s <= 512 for s in sizes), sizes
    return sizes


def _build_ffn_kernel(C):
    import concourse.mybir as mybir
    import concourse.tile as tile
    from concourse import bacc

    blocks = _token_blocks(C)
    starts = [sum(blocks[:i]) for i in range(len(blocks))]
    NTMAX = max(blocks)
    bf16 = mybir.dt.bfloat16
    f32 = mybir.dt.float32

    nc = bacc.Bacc("TRN2", target_bir_lowering=False, debug=False)
    # x is stored block-major: per partition, the [D//P, sz] slab of each
    # token block is contiguous, so each block is one clean DMA.
    xT = nc.dram_tensor("xT", [P, D // P * C], bf16, kind="ExternalInput").ap()
    w1t = nc.dram_tensor("w1t", [P, H // P, D // P, P], bf16, kind="ExternalInput").ap()
    w2t = nc.dram_tensor("w2t", [P, D // P, H // P, P], bf16, kind="ExternalInput").ap()
    b1t = nc.dram_tensor("b1t", [P, H // P], f32, kind="ExternalInput").ap()
    oT = nc.dram_tensor("oT", [P, D // P, C], f32, kind="ExternalOutput").ap()

    with tile.TileContext(nc) as tc:
        with (
            tc.tile_pool(name="const", bufs=1) as cpool,
            tc.tile_pool(name="resident", bufs=1) as rpool,
            tc.tile_pool(name="warm", bufs=1) as warmpool,
            tc.tile_pool(name="w1p", bufs=4) as w1pool,
            tc.tile_pool(name="w2p", bufs=3) as w2pool,
            tc.tile_pool(name="ost", bufs=4) as opool,
            tc.tile_pool(name="ps", bufs=4, space="PSUM") as pspool,
            tc.tile_pool(name="wps", bufs=1, space="PSUM") as wpspool,
        ):
            # PE warm-up: dependency-free matmuls run while input DMAs are in
            # flight, so the HAM clock-gate is already at 8/8 when real
            # matmuls start.  Zeroed input avoids NaN notifications.
            wsrc = warmpool.tile([P, 512], bf16)
            nc.gpsimd.memset(wsrc[:], 0.0)
            wps = wpspool.tile([P, 512], f32)
            for _ in range(5):
                nc.tensor.matmul(wps[:], wsrc[:, :P], wsrc[:], start=True, stop=True)

            # x arrives block-contiguous, split across BOTH HWDGE queues so
            # the first-needed transfers run in parallel:
            #   Sync  (Q1):  w1 slab 0, x block 1, w1 slabs 1..
            #   Scalar(Q10): x block 0, x block 2, b1, w2 prefetch
            w1_first = w1pool.tile([P, D // P, P], bf16, tag="w1s")
            nc.sync.dma_start(w1_first[:], w1t[:, 0])
            x_blks = []
            for blk, (st, sz) in enumerate(zip(starts, blocks)):
                xb = rpool.tile([P, D // P * sz], bf16, tag=f"xb{blk}")
                eng = nc.sync if blk == 1 else nc.scalar
                eng.dma_start(xb[:], xT[:, D // P * st : D // P * (st + sz)])
                x_blks.append(xb)
            b1_sb = cpool.tile([P, H // P], f32)
            nc.scalar.dma_start(b1_sb[:], b1t[:])
            h_sb = rpool.tile([P, H // P, C], bf16)

            # Layer 1: hT[:, ho, t] = gelu(sum_ko w1t[:,ho,ko,:].T @ xT[:,ko,t] + b1)
            for ho in range(H // P):
                if ho == 0:
                    w1s = w1_first
                else:
                    w1s = w1pool.tile([P, D // P, P], bf16, tag="w1s")
                    nc.sync.dma_start(w1s[:], w1t[:, ho])
                for blk, (st, sz) in enumerate(zip(starts, blocks)):
                    ps = pspool.tile([P, NTMAX], f32, tag="ps")
                    for ko in range(D // P):
                        nc.tensor.matmul(
                            ps[:, :sz],
                            w1s[:, ko, :],
                            x_blks[blk][:, ko * sz : (ko + 1) * sz],
                            start=(ko == 0),
                            stop=(ko == D // P - 1),
                        )
                    nc.scalar.activation(
                        h_sb[:, ho, st : st + sz],
                        ps[:, :sz],
                        mybir.ActivationFunctionType.Gelu,
                        bias=b1_sb[:, ho : ho + 1],
                    )

            # Layer 2: oT[:, do, t] = sum_ko w2t[:,do,ko,:].T @ hT[:,ko,t]
            for do in range(D // P):
                w2s = w2pool.tile([P, H // P, P], bf16, tag="w2s")
                nc.scalar.dma_start(w2s[:], w2t[:, do])
                for bi, (st, sz) in enumerate(zip(starts, blocks)):
                    ps = pspool.tile([P, NTMAX], f32, tag="ps")
                    for ko in range(H // P):
                        nc.tensor.matmul(
                            ps[:, :sz],
                            w2s[:, ko, :],
                            h_sb[:, ko, st : st + sz],
                            start=(ko == 0),
                            stop=(ko == H // P - 1),
                        )
                    last = do == D // P - 1 and bi == len(blocks) - 1
                    if not last:
                        ob = opool.tile([P, NTMAX], f32, tag="ob")
                        nc.vector.tensor_copy(ob[:, :sz], ps[:, :sz])
                        nc.scalar.dma_start(oT[:, do, st : st + sz], ob[:, :sz])
                    else:
                        # Final eviction is on the critical path: split it so
                        # the first half's DMA overlaps the second half's copy,
                        # using both HWDGE queues.
                        hsz = sz // 2
                        ob = opool.tile([P, NTMAX], f32, tag="ob")
                        nc.vector.tensor_copy(ob[:, :hsz], ps[:, :hsz])
                        nc.sync.dma_start(oT[:, do, st : st + hsz], ob[:, :hsz])
                        nc.vector.tensor_copy(ob[:, hsz:sz], ps[:, hsz:sz])
                        nc.scalar.dma_start(
                            oT[:, do, st + hsz : st + sz], ob[:, hsz:sz]
                        )

    nc.compile()
    return nc


def _route_host(x_flat, router_w):
    """Float64 router: returns per-expert (token_idx, combine_weight)."""
    logits = x_flat.astype(np.float64) @ router_w.astype(np.float64).T
    m = logits.max(axis=-1, keepdims=True)
    p = np.exp(logits - m)
    p /= p.sum(axis=-1, keepdims=True)
    order = np.argsort(-p, axis=-1)
    topi = order[:, :TOP_K]
    topw = np.take_along_axis(p, topi, axis=-1)
    topw /= topw.sum(axis=-1, keepdims=True)

    idx_list, wgt_list = [], []
    for e in range(E):
        mask = topi == e  # [T, TOP_K]; at most one True per row
        rows = np.nonzero(mask.any(axis=-1))[0]
        w = topw[rows][mask[rows]]
        idx_list.append(rows)
        wgt_list.append(w.astype(np.float32))
    return idx_list, wgt_list


def kernel(x, router_w, w1, b1, w2, b2):
    from concourse import bass_utils

    x = np.asarray(x)
    router_w = np.asarray(router_w)
    w1 = np.asarray(w1)
    b1 = np.asarray(b1)
    w2 = np.asarray(w2)
    b2 = np.asarray(b2)

    B, S, _ = x.shape
    T = B * S
    x_flat = x.reshape(T, D)

    idx_list, wgt_list = _route_host(x_flat, router_w)
    max_cnt = max(len(i) for i in idx_list)
    C = ((max_cnt + 31) // 32) * 32

    if C not in _COMPILED:
        _COMPILED[C] = _build_ffn_kernel(C)
    nc = _COMPILED[C]

    blocks = _token_blocks(C)
    starts = [sum(blocks[:i]) for i in range(len(blocks))]
    bf = ml_dtypes.bfloat16
    in_maps = []
    for e in range(E):
        idx = idx_list[e]
        n_e = len(idx)
        # xT [128, 8*C] block-major: pad tokens to C with zeros
        xg = np.zeros((C, D), np.float32)
        xg[:n_e] = x_flat[idx]
        xT_full = xg.T.reshape(D // P, P, C).transpose(1, 0, 2)  # [128, 8, C]
        xT_d = np.concatenate(
            [xT_full[:, :, st : st + sz].reshape(P, -1) for st, sz in zip(starts, blocks)],
            axis=1,
        ).astype(bf)
        w1_d = np.ascontiguousarray(
            w1[e].reshape(H // P, P, D // P, P).transpose(3, 0, 2, 1)
        ).astype(bf)
        w2_d = np.ascontiguousarray(
            w2[e].reshape(D // P, P, H // P, P).transpose(3, 0, 2, 1)
        ).astype(bf)
        b1_d = np.ascontiguousarray(b1[e].reshape(H // P, P).T).astype(np.float32)
        in_maps.append({"xT": xT_d, "w1t": w1_d, "w2t": w2_d, "b1t": b1_d})

    res = bass_utils.run_bass_kernel_spmd(nc, in_maps, core_ids=list(range(E)))

    out = np.zeros((T, D), np.float32)
    for e in range(E):
        idx = idx_list[e]
        n_e = len(idx)
        oT = res.results[e]["oT"]  # [128, 8, C]
        o_g = oT.transpose(1, 0, 2).reshape(D, C)[:, :n_e].T  # [n_e, D]
        out[idx] += wgt_list[e][:, None] * (o_g + b2[e][None, :])
    return out.reshape(B, S, D).astype(np.float32)
